# revision 39
# baseline (speedup 1.0000x reference)
"""Trainium2 Bass kernel for nn_Attention (B=2, T=2048, C=2048, H=16, causal, past_len=0).

Strategy: tensor-parallel over heads across 8 NeuronCores (2 heads/core).
  Phase 1 (qkv): each core computes q,k (transposed layout [hd, tok]) and v
    ([tok, hd]) for its 2 heads from the full token stream, in fp32r.
  Phase 2 (attention): per (batch, head): scoresT[k,q] = k.q/sqrt(hd) via PE,
    exp on ACT (no max-subtraction needed: scores are O(1)), causal mask by
    0/1 multiply on diagonal blocks, row-sums via a ones-matmul, out^T = v^T @
    attnT accumulated on PE, normalization by broadcasting 1/s across
    partitions.
  AllToAll: converts head-sharding -> token-sharding (each core ends up with
    all 16 heads' out^T for its 512 tokens). Two bf16 collectives (one per
    local head) so the first overlaps the second head's attention.
  Phase 3 (proj): y_slice[512, 2048] = out_slice @ proj_w.T computed locally
    in two passes (one per collective); proj weights are staged db-major so
    each 512-out-column block arrives as one tile: pass A's weights preload
    during phase 1 on the idle Pool DMA queue, pass B's during AllToAll#2.
    Host concatenates the 8 slices.

All matmul operands are fp32r (fp32 with low 12 mantissa bits rounded away)
= full PE rate, except the phase-3 stationary out-tiles and proj weights
(bf16, same PE rate, half the DMA/collective bytes).
"""
import sys
import numpy as np

if '/opt/trn_rl_repo' not in sys.path:
    sys.path.insert(0, '/opt/trn_rl_repo')

B, T, C, H, HD = 2, 2048, 2048, 16, 128
NCORES = 8
TOK = B * T            # 4096 global tokens
TSL = TOK // NCORES    # 512 tokens per core in the final output
SCALE = float(1.0 / np.sqrt(HD))

_CACHE = {}


def round_fp32r(x: np.ndarray) -> np.ndarray:
    """Round fp32 -> fp32r (drop low 12 mantissa bits, round-to-nearest-even)."""
    u = np.ascontiguousarray(x, dtype=np.float32).view(np.uint32)
    lsb = (u >> np.uint32(12)) & np.uint32(1)
    r = (u + np.uint32(0x7FF) + lsb) & np.uint32(0xFFFF_F000)
    return r.view(np.float32)


def build(debug_outputs=False):
    """Build the SPMD Bass program (same program on all 8 cores)."""
    import concourse.bacc as bacc
    import concourse.mybir as mybir
    from concourse import tile
    from contextlib import ExitStack

    f32 = mybir.dt.float32
    f32r = mybir.dt.float32r
    bf16 = mybir.dt.bfloat16
    Exp = mybir.ActivationFunctionType.Exp

    nc = bacc.Bacc("TRN2", target_bir_lowering=False, debug=False,
                   num_devices=NCORES)

    xT_d = nc.dram_tensor("xT", [C, TOK], f32r, kind="ExternalInput")
    wq_d = nc.dram_tensor("wqkvT", [C, 768], f32r, kind="ExternalInput")
    # proj weights, db-major: [pass, db, 8*128 rows, 512 cols] where pass 0 =
    # even global heads (local hl=0), pass 1 = odd heads; rows stack the 8
    # heads' 128 input dims; cols are the db'th 512 output dims.
    pwb_d = nc.dram_tensor("pwb", [2, 4, 8 * 128, 512], bf16,
                           kind="ExternalInput")
    masks_d = nc.dram_tensor("masks", [128, 128], bf16, kind="ExternalInput")
    ones_d = nc.dram_tensor("ones2", [128, 128], f32r, kind="ExternalInput")
    y_d = nc.dram_tensor("y", [TSL, C], f32, kind="ExternalOutput")
    if debug_outputs:
        dbg_qT = [nc.dram_tensor(f"dbg_qT{h}", [128, TOK], f32, kind="ExternalOutput") for h in range(2)]
        dbg_kT = [nc.dram_tensor(f"dbg_kT{h}", [128, TOK], f32, kind="ExternalOutput") for h in range(2)]
        dbg_v = nc.dram_tensor("dbg_v", [128, 32 * 256], f32, kind="ExternalOutput")

    with tile.TileContext(nc) as tc, ExitStack() as top:
        # ---- persistent pools
        sb_cst = top.enter_context(tc.tile_pool(name="cst", bufs=1))
        dram = top.enter_context(tc.tile_pool(name="dram", bufs=1, space="DRAM"))
        # phase-3 early pool sits below qkv on the pool stack so qkv can be
        # released before phase 3's late pool is created.
        sb_e3 = top.enter_context(tc.tile_pool(name="e3", bufs=1))
        pwA = [sb_e3.tile([128, 8 * 512], mybir.dt.bfloat16, name=f"pwA{db}",
                          tag=f"pwA{db}") for db in range(4)]
        otA = [sb_e3.tile([128, 512], mybir.dt.bfloat16, name=f"otA{m}",
                          tag=f"otA{m}") for m in range(8)]
        qkv_scope = top.enter_context(ExitStack())  # closed before phase 3
        sb_qkv = qkv_scope.enter_context(tc.tile_pool(name="qkv", bufs=1))

        qT = [sb_qkv.tile([128, TOK], f32r, name=f"qT{h}", tag=f"qT{h}") for h in range(2)]
        kT = [sb_qkv.tile([128, TOK], f32r, name=f"kT{h}", tag=f"kT{h}") for h in range(2)]
        v_sb = sb_qkv.tile([128, 32 * 256], f32r, name="v", tag="v")  # chunk ck at [:, ck*256:+256]

        # The causal boundary always crosses a diagonal 128x128 block as the
        # same lower-triangular (k_local <= q_local) pattern; columns left of
        # it are fully masked (skipped via PSUM sub-range accumulation) and
        # columns right of it are fully unmasked.
        mask_t = sb_cst.tile([128, 128], mybir.dt.bfloat16, name="masks",
                             tag="masks")
        ones_t = sb_cst.tile([128, 128], f32r, name="ones", tag="ones")

        a2a_in = [dram.tile([8 * 128, 512], bf16, name=f"ai{i}", tag=f"ai{i}") for i in range(2)]
        a2a_out = [dram.tile([8 * 128, 512], bf16, name=f"ao{i}", tag=f"ao{i}") for i in range(2)]

        # ================= Phase 1: qkv projection =================
        with ExitStack() as ph1, nc.named_scope("ph1_qkv"):
            sb_w = ph1.enter_context(tc.tile_pool(name="wq", bufs=1))
            sb_x = ph1.enter_context(tc.tile_pool(name="xs", bufs=5))
            ps_qk = ph1.enter_context(tc.tile_pool(name="pqk", bufs=1, space="PSUM"))
            ps_v = ph1.enter_context(tc.tile_pool(name="pv", bufs=1, space="PSUM"))

            wq_t = sb_w.tile([128, 16 * 768], f32r, name="wq", tag="wq")  # chunk c at [:, c*768:+768]

            for tb in range(8):  # 512-token blocks
                qk_ps = [ps_qk.tile([128, 512], f32, name=f"qk{f}", tag=f"qk{f}") for f in range(4)]
                v_ps = [ps_v.tile([128, 256], f32, name=f"v{s}", tag=f"v{s}") for s in range(4)]
                for cq in range(8):  # x loaded 2 c-chunks (512KB) per DMA
                    xt4 = sb_x.tile([128, 1024], f32r, name="xt4", tag="xt4")
                    if tb == 0:
                        # tb0: small per-chunk loads (low latency, spread
                        # across DMA queues) interleaved with weight chunks.
                        for cc in range(2):
                            c = 2 * cq + cc
                            for ws in range(3):
                                nc.sync.dma_start(
                                    wq_t[:, c * 768 + ws * 256:c * 768 + (ws + 1) * 256],
                                    wq_d[c * 128:(c + 1) * 128, ws * 256:(ws + 1) * 256])
                            for xs in range(2):
                                nc.sync.dma_start(
                                    xt4[:, cc * 512 + xs * 256:cc * 512 + (xs + 1) * 256],
                                    xT_d[c * 128:(c + 1) * 128, xs * 256:(xs + 1) * 256])
                    else:
                        for cc in range(2):  # two queues per c-chunk
                            c = 2 * cq + cc
                            for xs in range(2):
                                nc.sync.dma_start(
                                    xt4[:, cc * 512 + xs * 256:cc * 512 + (xs + 1) * 256],
                                    xT_d[c * 128:(c + 1) * 128,
                                         tb * 512 + xs * 256:tb * 512 + (xs + 1) * 256])
                    for cc in range(2):
                        c = 2 * cq + cc
                        xt = xt4[:, cc * 512:(cc + 1) * 512]
                        w_c = wq_t[:, c * 768:(c + 1) * 768]
                        for f in range(4):  # q_h0, q_h1, k_h0, k_h1
                            nc.tensor.matmul(qk_ps[f][:], w_c[:, f * 128:(f + 1) * 128],
                                             xt, start=(c == 0), stop=(c == 15))
                        for s in range(4):  # v for 128-token sub-chunks
                            nc.tensor.matmul(v_ps[s][:],
                                             xt[:, s * 128:(s + 1) * 128],
                                             w_c[:, 512:768],
                                             start=(c == 0), stop=(c == 15))
                sl = slice(tb * 512, (tb + 1) * 512)
                nc.scalar.copy(qT[0][:, sl], qk_ps[0][:])
                nc.vector.tensor_copy(kT[0][:, sl], qk_ps[2][:])
                nc.scalar.copy(qT[1][:, sl], qk_ps[1][:])
                nc.vector.tensor_copy(kT[1][:, sl], qk_ps[3][:])
                for s in range(4):
                    ck = tb * 4 + s
                    nc.vector.tensor_copy(v_sb[:, ck * 256:(ck + 1) * 256],
                                          v_ps[s][:])

        # pass-A proj weights prefetch: issued on the SP queue after phase
        # 1's triggers, so the transfers don't contend with the startup
        # x/w loads but still land long before pass A needs them.
        for db in range(4):
            nc.sync.dma_start(
                pwA[db][:].rearrange("p (c w) -> p c w", c=8),
                pwb_d[0, db].rearrange("(c p) w -> p c w", p=128))

        # ================= Phase 2: attention =================
        with ExitStack() as ph2, nc.named_scope("ph2_attn"):
            ps_sc = ph2.enter_context(tc.tile_pool(name="psc", bufs=3, space="PSUM"))
            ps_o = ph2.enter_context(tc.tile_pool(name="po", bufs=3, space="PSUM"))
            ps_s = ph2.enter_context(tc.tile_pool(name="pss", bufs=2, space="PSUM"))
            sb_et = ph2.enter_context(tc.tile_pool(name="et", bufs=18))
            sb_sm = ph2.enter_context(tc.tile_pool(name="sm", bufs=3))
            sb_on = ph2.enter_context(tc.tile_pool(name="on", bufs=4))

            nc.sync.dma_start(mask_t[:], masks_d[:])
            nc.sync.dma_start(ones_t[:], ones_d[:])

            for idx, (b, hl) in enumerate([(0, 0), (1, 0), (0, 1), (1, 1)]):
                qTb = qT[hl][:, b * T:(b + 1) * T]
                kTb = kT[hl][:, b * T:(b + 1) * T]
                for g in range(4):  # query groups of 512
                    nk = 4 * (g + 1)
                    o_ps = ps_o.tile([128, 512], f32, name="o", tag="o")
                    # ones lhsT is [128,128]: every output partition gets the
                    # k-sum, i.e. the softmax denominator pre-broadcast.
                    s_ps = ps_s.tile([128, 512], f32, name="s", tag="s")
                    # Diagonal (masked) blocks first, and o-matmuls lag the
                    # sc-matmuls by 2 blocks, so the exp+mask latency hides
                    # behind the next blocks' sc work instead of stalling PE.
                    # Diagonal (masked) blocks first -- the first one (lo=0)
                    # covers the full 512 columns, so its start=True matmul
                    # initializes the whole o/s PSUM range; later diagonal
                    # blocks touch only their valid column suffix [lo:512].
                    # o-matmuls lag the sc-matmuls by 2 blocks so the
                    # exp+mask latency hides behind the next blocks' sc work.
                    order = list(range(4 * g, nk)) + list(range(4 * g))
                    LAG = 2
                    ets = []

                    def emit_o(j, last):
                        kj, et, lo = ets[j]
                        ck = b * 16 + kj
                        nc.tensor.matmul(o_ps[:, lo:512],
                                         v_sb[:, ck * 256 + hl * 128:ck * 256 + (hl + 1) * 128],
                                         et[:, lo:512], start=(j == 0), stop=last)

                    for i, kj in enumerate(order):
                        lo = (kj - 4 * g) * 128 if kj >= 4 * g else 0
                        sc_ps = ps_sc.tile([128, 512], f32, name="sc", tag="sc")
                        et = sb_et.tile([128, 512], f32r, name="et", tag="et")
                        nc.tensor.matmul(sc_ps[:, lo:512], kTb[:, kj * 128:(kj + 1) * 128],
                                         qTb[:, g * 512 + lo:(g + 1) * 512],
                                         start=True, stop=True)
                        nc.scalar.activation(et[:, lo:512], sc_ps[:, lo:512],
                                             Exp, scale=SCALE)
                        if kj >= 4 * g:  # diagonal block: causal tri mask
                            nc.vector.tensor_mul(et[:, lo:lo + 128],
                                                 et[:, lo:lo + 128], mask_t[:])
                        ets.append((kj, et, lo))
                        if i >= LAG:
                            emit_o(i - LAG, last=False)
                    for j in range(max(0, nk - LAG), nk):
                        emit_o(j, last=(j == nk - 1))
                    # s-matmuls batched: consecutive mms share the ones
                    # stationary (no v/kT weight reloads interleaved)
                    for j, (kj, et, lo) in enumerate(ets):
                        nc.tensor.matmul(s_ps[:, lo:512], ones_t[:], et[:, lo:512],
                                         start=(j == 0), stop=(j == nk - 1))
                    rs_bc = sb_sm.tile([128, 512], f32, name="rs_bc", tag="rs_bc")
                    nc.vector.reciprocal(rs_bc[:], s_ps[:])
                    on = sb_on.tile([128, 512], bf16, name="on", tag="on")
                    nc.vector.tensor_mul(on[:], o_ps[:], rs_bc[:])
                    dest = b * 4 + g
                    nc.sync.dma_start(a2a_in[hl][dest * 128:(dest + 1) * 128, :],
                                      on[:])
                if idx in (1, 3):  # both batches of this local head done
                    nc.gpsimd.collective_compute(
                        "AllToAll", mybir.AluOpType.bypass,
                        replica_groups=[list(range(NCORES))],
                        ins=[a2a_in[hl].opt()], outs=[a2a_out[hl].opt()],
                    )
                    if idx == 3:
                        # pass-A ot loads: on the Pool queue AFTER both
                        # collective triggers (so they can't delay A2A#2);
                        # the descriptors fire the moment A2A#1 completes.
                        for m in range(8):
                            nc.gpsimd.dma_start(
                                otA[m][:], a2a_out[0][m * 128:(m + 1) * 128, :])

        if debug_outputs:
            for h in range(2):
                nc.sync.dma_start(dbg_qT[h][:], qT[h][:].bitcast(f32))
                nc.sync.dma_start(dbg_kT[h][:], kT[h][:].bitcast(f32))
            nc.sync.dma_start(dbg_v[:], v_sb[:].bitcast(f32))

        # ================= Phase 3: output projection =================
        qkv_scope.close()  # release qT/kT/v SBUF for phase 3's late pool
        with ExitStack() as ph3, nc.named_scope("ph3_proj"):
            sb_l3 = ph3.enter_context(tc.tile_pool(name="l3", bufs=1))
            sb_y = ph3.enter_context(tc.tile_pool(name="ysb", bufs=3))
            ps_y = ph3.enter_context(tc.tile_pool(name="py", bufs=1, space="PSUM"))
            ps_yb = ph3.enter_context(tc.tile_pool(name="pyb", bufs=1, space="PSUM"))

            # pass-B weights load during A2A#2 (their SBUF frees when
            # attention's last matmuls retire); db-major so db=0 lands first.
            pwB = []
            for db in range(4):
                pw = sb_l3.tile([128, 8 * 512], bf16, name=f"pwB{db}", tag=f"pwB{db}")
                nc.gpsimd.dma_start(
                    pw[:].rearrange("p (c w) -> p c w", c=8),
                    pwb_d[1, db].rearrange("(c p) w -> p c w", p=128))
                pwB.append(pw)
            otB = []
            for m in range(8):
                ot = sb_l3.tile([128, 512], bf16, name=f"otB{m}", tag=f"otB{m}")
                nc.gpsimd.dma_start(ot[:], a2a_out[1][m * 128:(m + 1) * 128, :])
                otB.append(ot)
            y_acc = sb_l3.tile([128, 4 * 4 * 512], f32, name="yacc", tag="yacc")

            # Pass A: heads from A2A#1 -> SBUF partial, while A2A#2 flies.
            for db in range(4):  # 512-wide output column blocks
                y_ps = [ps_y.tile([128, 512], f32, name=f"y{t_}", tag=f"y{t_}")
                        for t_ in range(4)]
                for mi in range(8):
                    for t_ in range(4):
                        nc.tensor.matmul(y_ps[t_][:], otA[mi][:, t_ * 128:(t_ + 1) * 128],
                                         pwA[db][:, mi * 512:(mi + 1) * 512],
                                         start=(mi == 0), stop=(mi == 7))
                for t_ in range(4):
                    acc = y_acc[:, (db * 4 + t_) * 512:(db * 4 + t_ + 1) * 512]
                    nc.scalar.copy(acc, y_ps[t_][:])
            # Pass B: add the A2A#2 heads, emit y.
            for db in range(4):
                y_ps = [ps_yb.tile([128, 512], f32, name=f"yB{t_}", tag=f"yB{t_}")
                        for t_ in range(4)]
                for mi in range(8):
                    for t_ in range(4):
                        nc.tensor.matmul(y_ps[t_][:], otB[mi][:, t_ * 128:(t_ + 1) * 128],
                                         pwB[db][:, mi * 512:(mi + 1) * 512],
                                         start=(mi == 0), stop=(mi == 7))
                for t_ in range(4):
                    acc = y_acc[:, (db * 4 + t_) * 512:(db * 4 + t_ + 1) * 512]
                    y_sb = sb_y.tile([128, 512], f32, name="ysb", tag="ysb")
                    nc.vector.tensor_add(y_sb[:], y_ps[t_][:], acc)
                    for yh in range(2):  # two queues for the writeback
                        nc.sync.dma_start(
                            y_d[t_ * 128:(t_ + 1) * 128,
                                db * 512 + yh * 256:db * 512 + (yh + 1) * 256],
                            y_sb[:, yh * 256:(yh + 1) * 256])

    nc.finalize()
    return nc


def prep_in_maps(x, qkv_w, proj_w):
    """Host-side sharding + fp32r pre-rounding. Returns per-core input maps."""
    import ml_dtypes

    x = np.ascontiguousarray(np.asarray(x, dtype=np.float32).reshape(TOK, C))
    qkv_w = np.asarray(qkv_w, dtype=np.float32)
    proj_w = np.asarray(proj_w, dtype=np.float32)

    xT = round_fp32r(x.T)                       # [C, TOK], shared
    pwT = proj_w.T                              # [C, C]
    # db-major bf16 staging: [pass, db, 8*128, 512]; pass 0 = even heads.
    pwT4 = pwT.reshape(16, 128, 4, 512)
    pwb = np.stack([
        pwT4[0::2].transpose(2, 0, 1, 3).reshape(4, 8 * 128, 512),
        pwT4[1::2].transpose(2, 0, 1, 3).reshape(4, 8 * 128, 512),
    ]).astype(ml_dtypes.bfloat16)
    # lower-triangular diagonal-block mask (k_local <= q_local), 0/1 in bf16
    masks = (np.arange(128)[:, None] <= np.arange(128)[None, :]).astype(
        ml_dtypes.bfloat16)
    ones2 = np.ones((128, 128), dtype=np.float32)

    in_maps = []
    for i in range(NCORES):
        r0 = 2 * i * HD
        rows = np.concatenate([
            qkv_w[r0:r0 + 2 * HD],              # q rows, heads 2i, 2i+1
            qkv_w[C + r0:C + r0 + 2 * HD],      # k rows
            qkv_w[2 * C + r0:2 * C + r0 + 2 * HD],  # v rows
        ], axis=0)                              # [768, C]
        wqkvT = round_fp32r(rows.T)             # [C, 768]
        in_maps.append({"xT": xT, "wqkvT": wqkvT, "pwb": pwb,
                        "masks": masks, "ones2": ones2})
    return in_maps


def kernel(x, qkv_w, proj_w, past=None, past_len=0, **_ignored):
    # past is fully overwritten before being read (past_len == 0), so the
    # output does not depend on it.
    from concourse.bass_utils import run_bass_kernel_spmd
    nc = _CACHE.get("nc")
    if nc is None:
        nc = _CACHE["nc"] = build()
    in_maps = prep_in_maps(x, qkv_w, proj_w)
    res = run_bass_kernel_spmd(nc, in_maps, list(range(NCORES)))
    y = np.concatenate([res.results[i]["y"] for i in range(NCORES)], axis=0)
    return np.ascontiguousarray(y.reshape(B, T, C), dtype=np.float32)


# revision 41
# speedup vs baseline: 1.0847x; 1.0847x over previous
"""Trainium2 Bass kernel for nn_Attention (B=2, T=2048, C=2048, H=16, causal, past_len=0).

Strategy: tensor-parallel over heads across 8 NeuronCores (2 heads/core).
  Phase 1 (qkv): each core computes q,k (transposed layout [hd, tok]) and v
    ([tok, hd]) for its 2 heads from the full token stream, in fp32r.
  Phase 2 (attention): per (batch, head): scoresT[k,q] = k.q/sqrt(hd) via PE,
    exp on ACT (no max-subtraction needed: scores are O(1)), causal mask by
    0/1 multiply on diagonal blocks, row-sums via a ones-matmul, out^T = v^T @
    attnT accumulated on PE, normalization by broadcasting 1/s across
    partitions.
  AllToAll: converts head-sharding -> token-sharding (each core ends up with
    all 16 heads' out^T for its 512 tokens). Two bf16 collectives (one per
    local head) so the first overlaps the second head's attention.
  Phase 3 (proj): y_slice[512, 2048] = out_slice @ proj_w.T computed locally
    in two passes (one per collective); proj weights are staged db-major so
    each 512-out-column block arrives as one tile: pass A's weights preload
    during phase 1 on the idle Pool DMA queue, pass B's during AllToAll#2.
    Host concatenates the 8 slices.

All matmul operands are fp32r (fp32 with low 12 mantissa bits rounded away)
= full PE rate, except the phase-3 stationary out-tiles and proj weights
(bf16, same PE rate, half the DMA/collective bytes).
"""
import sys
import numpy as np

if '/opt/trn_rl_repo' not in sys.path:
    sys.path.insert(0, '/opt/trn_rl_repo')

B, T, C, H, HD = 2, 2048, 2048, 16, 128
NCORES = 8
TOK = B * T            # 4096 global tokens
TSL = TOK // NCORES    # 512 tokens per core in the final output
SCALE = float(1.0 / np.sqrt(HD))

_CACHE = {}


def round_fp32r(x: np.ndarray) -> np.ndarray:
    """Round fp32 -> fp32r (drop low 12 mantissa bits, round-to-nearest-even)."""
    u = np.ascontiguousarray(x, dtype=np.float32).view(np.uint32)
    lsb = (u >> np.uint32(12)) & np.uint32(1)
    r = (u + np.uint32(0x7FF) + lsb) & np.uint32(0xFFFF_F000)
    return r.view(np.float32)


def build(debug_outputs=False):
    """Build the SPMD Bass program (same program on all 8 cores)."""
    import concourse.bacc as bacc
    import concourse.mybir as mybir
    from concourse import tile
    from contextlib import ExitStack

    f32 = mybir.dt.float32
    f32r = mybir.dt.float32r
    bf16 = mybir.dt.bfloat16
    Exp = mybir.ActivationFunctionType.Exp

    nc = bacc.Bacc("TRN2", target_bir_lowering=False, debug=False,
                   num_devices=NCORES)

    xT_d = nc.dram_tensor("xT", [C, TOK], f32r, kind="ExternalInput")
    wq_d = nc.dram_tensor("wqkvT", [C, 768], f32r, kind="ExternalInput")
    # proj weights, db-major: [pass, db, 8*128 rows, 512 cols] where pass 0 =
    # even global heads (local hl=0), pass 1 = odd heads; rows stack the 8
    # heads' 128 input dims; cols are the db'th 512 output dims.
    pwb_d = nc.dram_tensor("pwb", [2, 4, 8 * 128, 512], bf16,
                           kind="ExternalInput")
    masks_d = nc.dram_tensor("masks", [128, 128], bf16, kind="ExternalInput")
    ones_d = nc.dram_tensor("ones2", [128, 128], f32r, kind="ExternalInput")
    y_d = nc.dram_tensor("y", [TSL, C], f32, kind="ExternalOutput")
    if debug_outputs:
        dbg_qT = [nc.dram_tensor(f"dbg_qT{h}", [128, TOK], f32, kind="ExternalOutput") for h in range(2)]
        dbg_kT = [nc.dram_tensor(f"dbg_kT{h}", [128, TOK], f32, kind="ExternalOutput") for h in range(2)]
        dbg_v = nc.dram_tensor("dbg_v", [128, 32 * 256], f32, kind="ExternalOutput")

    with tile.TileContext(nc) as tc, ExitStack() as top:
        # ---- persistent pools
        sb_cst = top.enter_context(tc.tile_pool(name="cst", bufs=1))
        dram = top.enter_context(tc.tile_pool(name="dram", bufs=1, space="DRAM"))
        # phase-3 early pool sits below qkv on the pool stack so qkv can be
        # released before phase 3's late pool is created.
        sb_e3 = top.enter_context(tc.tile_pool(name="e3", bufs=1))
        pwA = [sb_e3.tile([128, 8 * 512], mybir.dt.bfloat16, name=f"pwA{db}",
                          tag=f"pwA{db}") for db in range(4)]
        otA = [sb_e3.tile([128, 512], mybir.dt.bfloat16, name=f"otA{m}",
                          tag=f"otA{m}") for m in range(8)]
        qkv_scope = top.enter_context(ExitStack())  # closed before phase 3
        sb_qkv = qkv_scope.enter_context(tc.tile_pool(name="qkv", bufs=1))

        qT = [sb_qkv.tile([128, TOK], f32r, name=f"qT{h}", tag=f"qT{h}") for h in range(2)]
        kT = [sb_qkv.tile([128, TOK], f32r, name=f"kT{h}", tag=f"kT{h}") for h in range(2)]
        v_sb = sb_qkv.tile([128, 32 * 256], f32r, name="v", tag="v")  # chunk ck at [:, ck*256:+256]

        # The causal boundary always crosses a diagonal 128x128 block as the
        # same lower-triangular (k_local <= q_local) pattern; columns left of
        # it are fully masked (skipped via PSUM sub-range accumulation) and
        # columns right of it are fully unmasked.
        mask_t = sb_cst.tile([128, 128], mybir.dt.bfloat16, name="masks",
                             tag="masks")
        ones_t = sb_cst.tile([128, 128], f32r, name="ones", tag="ones")

        a2a_in = [dram.tile([8 * 128, 512], bf16, name=f"ai{i}", tag=f"ai{i}") for i in range(2)]
        a2a_out = [dram.tile([8 * 128, 512], bf16, name=f"ao{i}", tag=f"ao{i}") for i in range(2)]

        # ================= Phase 1: qkv projection =================
        with ExitStack() as ph1, nc.named_scope("ph1_qkv"):
            sb_w = ph1.enter_context(tc.tile_pool(name="wq", bufs=1))
            sb_x = ph1.enter_context(tc.tile_pool(name="xs", bufs=5))
            ps_qk = ph1.enter_context(tc.tile_pool(name="pqk", bufs=1, space="PSUM"))
            ps_v = ph1.enter_context(tc.tile_pool(name="pv", bufs=1, space="PSUM"))

            wq_t = sb_w.tile([128, 16 * 768], f32r, name="wq", tag="wq")  # chunk c at [:, c*768:+768]

            for tb in range(8):  # 512-token blocks
                qk_ps = [ps_qk.tile([128, 512], f32, name=f"qk{f}", tag=f"qk{f}") for f in range(4)]
                v_ps = [ps_v.tile([128, 256], f32, name=f"v{s}", tag=f"v{s}") for s in range(4)]
                for cq in range(8):  # x loaded 2 c-chunks (512KB) per DMA
                    xt4 = sb_x.tile([128, 1024], f32r, name="xt4", tag="xt4")
                    if tb == 0:
                        # tb0: small per-chunk loads (low latency, spread
                        # across DMA queues) interleaved with weight chunks.
                        for cc in range(2):
                            c = 2 * cq + cc
                            for ws in range(3):
                                nc.sync.dma_start(
                                    wq_t[:, c * 768 + ws * 256:c * 768 + (ws + 1) * 256],
                                    wq_d[c * 128:(c + 1) * 128, ws * 256:(ws + 1) * 256])
                            for xs in range(2):
                                nc.sync.dma_start(
                                    xt4[:, cc * 512 + xs * 256:cc * 512 + (xs + 1) * 256],
                                    xT_d[c * 128:(c + 1) * 128, xs * 256:(xs + 1) * 256])
                    else:
                        for cc in range(2):  # two queues per c-chunk
                            c = 2 * cq + cc
                            for xs in range(2):
                                nc.sync.dma_start(
                                    xt4[:, cc * 512 + xs * 256:cc * 512 + (xs + 1) * 256],
                                    xT_d[c * 128:(c + 1) * 128,
                                         tb * 512 + xs * 256:tb * 512 + (xs + 1) * 256])
                    for cc in range(2):
                        c = 2 * cq + cc
                        xt = xt4[:, cc * 512:(cc + 1) * 512]
                        w_c = wq_t[:, c * 768:(c + 1) * 768]
                        for f in range(4):  # q_h0, q_h1, k_h0, k_h1
                            nc.tensor.matmul(qk_ps[f][:], w_c[:, f * 128:(f + 1) * 128],
                                             xt, start=(c == 0), stop=(c == 15))
                        for s in range(4):  # v for 128-token sub-chunks
                            nc.tensor.matmul(v_ps[s][:],
                                             xt[:, s * 128:(s + 1) * 128],
                                             w_c[:, 512:768],
                                             start=(c == 0), stop=(c == 15))
                sl = slice(tb * 512, (tb + 1) * 512)
                nc.scalar.copy(qT[0][:, sl], qk_ps[0][:])
                nc.vector.tensor_copy(kT[0][:, sl], qk_ps[2][:])
                nc.scalar.copy(qT[1][:, sl], qk_ps[1][:])
                nc.vector.tensor_copy(kT[1][:, sl], qk_ps[3][:])
                for s in range(4):
                    ck = tb * 4 + s
                    nc.vector.tensor_copy(v_sb[:, ck * 256:(ck + 1) * 256],
                                          v_ps[s][:])

        # pass-A proj weights prefetch: issued on the SP queue after phase
        # 1's triggers, so the transfers don't contend with the startup
        # x/w loads but still land long before pass A needs them.
        for db in range(4):
            nc.sync.dma_start(
                pwA[db][:].rearrange("p (c w) -> p c w", c=8),
                pwb_d[0, db].rearrange("(c p) w -> p c w", p=128))

        # ================= Phase 2: attention =================
        with ExitStack() as ph2, nc.named_scope("ph2_attn"):
            ps_sc = ph2.enter_context(tc.tile_pool(name="psc", bufs=3, space="PSUM"))
            ps_o = ph2.enter_context(tc.tile_pool(name="po", bufs=3, space="PSUM"))
            ps_s = ph2.enter_context(tc.tile_pool(name="pss", bufs=2, space="PSUM"))
            sb_et = ph2.enter_context(tc.tile_pool(name="et", bufs=22))
            sb_sm = ph2.enter_context(tc.tile_pool(name="sm", bufs=3))
            sb_on = ph2.enter_context(tc.tile_pool(name="on", bufs=4))

            nc.sync.dma_start(mask_t[:], masks_d[:])
            nc.sync.dma_start(ones_t[:], ones_d[:])

            # Software pipeline across query groups: each group's finalize
            # (s-matmuls, reciprocal, normalize, A2A-input DMA) is deferred
            # until after the NEXT group's diagonal section has been emitted,
            # so the DVE runs the next masks before the recip/normalize and
            # the o-matmuls never wait on masks at a group boundary.
            pending = [None]

            def flush():
                if pending[0] is not None:
                    pending[0]()
                    pending[0] = None

            for idx, (b, hl) in enumerate([(0, 0), (1, 0), (0, 1), (1, 1)]):
                qTb = qT[hl][:, b * T:(b + 1) * T]
                kTb = kT[hl][:, b * T:(b + 1) * T]
                for g in range(4):  # query groups of 512
                    nk = 4 * (g + 1)
                    o_ps = ps_o.tile([128, 512], f32, name="o", tag="o")
                    # ones lhsT is [128,128]: every output partition gets the
                    # k-sum, i.e. the softmax denominator pre-broadcast.
                    s_ps = ps_s.tile([128, 512], f32, name="s", tag="s")
                    # Diagonal (masked) blocks first -- the first one (lo=0)
                    # covers the full 512 columns, so its start=True matmul
                    # initializes the whole o/s PSUM range; later diagonal
                    # blocks touch only their valid column suffix [lo:512].
                    # o-matmuls lag the sc-matmuls by 2 blocks so the
                    # exp+mask latency hides behind the next blocks' sc work.
                    order = list(range(4 * g, nk)) + list(range(4 * g))
                    LAG = 2
                    ets = []

                    def emit_o(j, last, o_ps=o_ps, ets=ets, b=b, hl=hl):
                        kj, et, lo = ets[j]
                        ck = b * 16 + kj
                        nc.tensor.matmul(o_ps[:, lo:512],
                                         v_sb[:, ck * 256 + hl * 128:ck * 256 + (hl + 1) * 128],
                                         et[:, lo:512], start=(j == 0), stop=last)

                    for i, kj in enumerate(order):
                        lo = (kj - 4 * g) * 128 if kj >= 4 * g else 0
                        sc_ps = ps_sc.tile([128, 512], f32, name="sc", tag="sc")
                        et = sb_et.tile([128, 512], f32r, name="et", tag="et")
                        nc.tensor.matmul(sc_ps[:, lo:512], kTb[:, kj * 128:(kj + 1) * 128],
                                         qTb[:, g * 512 + lo:(g + 1) * 512],
                                         start=True, stop=True)
                        nc.scalar.activation(et[:, lo:512], sc_ps[:, lo:512],
                                             Exp, scale=SCALE)
                        if kj >= 4 * g:  # diagonal block: causal tri mask
                            nc.vector.tensor_mul(et[:, lo:lo + 128],
                                                 et[:, lo:lo + 128], mask_t[:])
                        ets.append((kj, et, lo))
                        if i == 3:  # diagonal section done: prev group's turn
                            flush()
                        if i >= LAG:
                            emit_o(i - LAG, last=False)
                    for j in range(max(0, nk - LAG), nk):
                        emit_o(j, last=(j == nk - 1))

                    def finalize(ets=ets, s_ps=s_ps, o_ps=o_ps, nk=nk,
                                 b=b, hl=hl, g=g):
                        # s-matmuls batched: consecutive mms share the ones
                        # stationary (no v/kT weight reloads interleaved)
                        for j, (kj, et, lo) in enumerate(ets):
                            nc.tensor.matmul(s_ps[:, lo:512], ones_t[:],
                                             et[:, lo:512],
                                             start=(j == 0), stop=(j == nk - 1))
                        rs_bc = sb_sm.tile([128, 512], f32, name="rs_bc", tag="rs_bc")
                        nc.vector.reciprocal(rs_bc[:], s_ps[:])
                        on = sb_on.tile([128, 512], bf16, name="on", tag="on")
                        nc.vector.tensor_mul(on[:], o_ps[:], rs_bc[:])
                        dest = b * 4 + g
                        nc.sync.dma_start(a2a_in[hl][dest * 128:(dest + 1) * 128, :],
                                          on[:])

                    pending[0] = finalize
                if idx in (1, 3):  # both batches of this local head done
                    flush()
                    nc.gpsimd.collective_compute(
                        "AllToAll", mybir.AluOpType.bypass,
                        replica_groups=[list(range(NCORES))],
                        ins=[a2a_in[hl].opt()], outs=[a2a_out[hl].opt()],
                    )
                    if idx == 3:
                        # pass-A ot loads: on the Pool queue AFTER both
                        # collective triggers (so they can't delay A2A#2);
                        # the descriptors fire the moment A2A#1 completes.
                        for m in range(8):
                            nc.gpsimd.dma_start(
                                otA[m][:], a2a_out[0][m * 128:(m + 1) * 128, :])

        if debug_outputs:
            for h in range(2):
                nc.sync.dma_start(dbg_qT[h][:], qT[h][:].bitcast(f32))
                nc.sync.dma_start(dbg_kT[h][:], kT[h][:].bitcast(f32))
            nc.sync.dma_start(dbg_v[:], v_sb[:].bitcast(f32))

        # ================= Phase 3: output projection =================
        qkv_scope.close()  # release qT/kT/v SBUF for phase 3's late pool
        with ExitStack() as ph3, nc.named_scope("ph3_proj"):
            sb_l3 = ph3.enter_context(tc.tile_pool(name="l3", bufs=1))
            sb_y = ph3.enter_context(tc.tile_pool(name="ysb", bufs=3))
            ps_y = ph3.enter_context(tc.tile_pool(name="py", bufs=1, space="PSUM"))
            ps_yb = ph3.enter_context(tc.tile_pool(name="pyb", bufs=1, space="PSUM"))

            # pass-B weights load during A2A#2 (their SBUF frees when
            # attention's last matmuls retire); db-major so db=0 lands first.
            pwB = []
            for db in range(4):
                pw = sb_l3.tile([128, 8 * 512], bf16, name=f"pwB{db}", tag=f"pwB{db}")
                nc.gpsimd.dma_start(
                    pw[:].rearrange("p (c w) -> p c w", c=8),
                    pwb_d[1, db].rearrange("(c p) w -> p c w", p=128))
                pwB.append(pw)
            otB = []
            for m in range(8):
                ot = sb_l3.tile([128, 512], bf16, name=f"otB{m}", tag=f"otB{m}")
                nc.gpsimd.dma_start(ot[:], a2a_out[1][m * 128:(m + 1) * 128, :])
                otB.append(ot)
            y_acc = sb_l3.tile([128, 4 * 4 * 512], f32, name="yacc", tag="yacc")

            # Pass A: heads from A2A#1 -> SBUF partial, while A2A#2 flies.
            for db in range(4):  # 512-wide output column blocks
                y_ps = [ps_y.tile([128, 512], f32, name=f"y{t_}", tag=f"y{t_}")
                        for t_ in range(4)]
                for mi in range(8):
                    for t_ in range(4):
                        nc.tensor.matmul(y_ps[t_][:], otA[mi][:, t_ * 128:(t_ + 1) * 128],
                                         pwA[db][:, mi * 512:(mi + 1) * 512],
                                         start=(mi == 0), stop=(mi == 7))
                for t_ in range(4):
                    acc = y_acc[:, (db * 4 + t_) * 512:(db * 4 + t_ + 1) * 512]
                    nc.scalar.copy(acc, y_ps[t_][:])
            # Pass B: add the A2A#2 heads, emit y.
            for db in range(4):
                y_ps = [ps_yb.tile([128, 512], f32, name=f"yB{t_}", tag=f"yB{t_}")
                        for t_ in range(4)]
                for mi in range(8):
                    for t_ in range(4):
                        nc.tensor.matmul(y_ps[t_][:], otB[mi][:, t_ * 128:(t_ + 1) * 128],
                                         pwB[db][:, mi * 512:(mi + 1) * 512],
                                         start=(mi == 0), stop=(mi == 7))
                for t_ in range(4):
                    acc = y_acc[:, (db * 4 + t_) * 512:(db * 4 + t_ + 1) * 512]
                    y_sb = sb_y.tile([128, 512], f32, name="ysb", tag="ysb")
                    nc.vector.tensor_add(y_sb[:], y_ps[t_][:], acc)
                    for yh in range(2):  # two queues for the writeback
                        nc.sync.dma_start(
                            y_d[t_ * 128:(t_ + 1) * 128,
                                db * 512 + yh * 256:db * 512 + (yh + 1) * 256],
                            y_sb[:, yh * 256:(yh + 1) * 256])

    nc.finalize()
    return nc


def prep_in_maps(x, qkv_w, proj_w):
    """Host-side sharding + fp32r pre-rounding. Returns per-core input maps."""
    import ml_dtypes

    x = np.ascontiguousarray(np.asarray(x, dtype=np.float32).reshape(TOK, C))
    qkv_w = np.asarray(qkv_w, dtype=np.float32)
    proj_w = np.asarray(proj_w, dtype=np.float32)

    xT = round_fp32r(x.T)                       # [C, TOK], shared
    pwT = proj_w.T                              # [C, C]
    # db-major bf16 staging: [pass, db, 8*128, 512]; pass 0 = even heads.
    pwT4 = pwT.reshape(16, 128, 4, 512)
    pwb = np.stack([
        pwT4[0::2].transpose(2, 0, 1, 3).reshape(4, 8 * 128, 512),
        pwT4[1::2].transpose(2, 0, 1, 3).reshape(4, 8 * 128, 512),
    ]).astype(ml_dtypes.bfloat16)
    # lower-triangular diagonal-block mask (k_local <= q_local), 0/1 in bf16
    masks = (np.arange(128)[:, None] <= np.arange(128)[None, :]).astype(
        ml_dtypes.bfloat16)
    ones2 = np.ones((128, 128), dtype=np.float32)

    in_maps = []
    for i in range(NCORES):
        r0 = 2 * i * HD
        rows = np.concatenate([
            qkv_w[r0:r0 + 2 * HD],              # q rows, heads 2i, 2i+1
            qkv_w[C + r0:C + r0 + 2 * HD],      # k rows
            qkv_w[2 * C + r0:2 * C + r0 + 2 * HD],  # v rows
        ], axis=0)                              # [768, C]
        wqkvT = round_fp32r(rows.T)             # [C, 768]
        in_maps.append({"xT": xT, "wqkvT": wqkvT, "pwb": pwb,
                        "masks": masks, "ones2": ones2})
    return in_maps


def kernel(x, qkv_w, proj_w, past=None, past_len=0, **_ignored):
    # past is fully overwritten before being read (past_len == 0), so the
    # output does not depend on it.
    from concourse.bass_utils import run_bass_kernel_spmd
    nc = _CACHE.get("nc")
    if nc is None:
        nc = _CACHE["nc"] = build()
    in_maps = prep_in_maps(x, qkv_w, proj_w)
    res = run_bass_kernel_spmd(nc, in_maps, list(range(NCORES)))
    y = np.concatenate([res.results[i]["y"] for i in range(NCORES)], axis=0)
    return np.ascontiguousarray(y.reshape(B, T, C), dtype=np.float32)
